# revision 1
# baseline (speedup 1.0000x reference)
"""Multi-head attention (B=2, T=2048, D=512, H=8) on 8 trn2 NeuronCores.

Sharding: data + head parallel. Core c handles batch b = c//4 and head pair
p = c%4 (heads 2p, 2p+1 <-> feature rows 128p .. 128p+127 of the 512-wide
projection space).  Each core:
  - projects its 2 heads' q/k (layout [feat, tok], feat on partitions) and
    v (layout [tok, feat]) from host-pre-transposed bf16 inputs,
  - computes scoresT = k_h q_h^T in [key, query] orientation (keys on
    partitions) with row-tiled head pairs,
  - softmax exp split across TWO engines: most tiles on ACT (hardware exp,
    scaled by 1/sqrt(dk); no max subtraction, |scores| <~ 10), a tuned
    subset on the DVE as an averaged double-Schraudolph: bits = round(
    a*score + b) saturating-cast to int16 are bf16 exp2 bits; the pair
    {S(y), S(y+0.5)} (the second is exactly bits+64) brackets the sawtooth
    error (+-1.8% after mean calibration) and is summed for free by PV
    psum accumulation,
  - PV in [query, feat] orientation: stationary = ex q-tile [k, q], moving
    = v_aug [k, 65] (v plus a ones column so the softmax denominator
    accumulates as an extra psum column).  Full 128-wide PE contraction
    and 128 output partitions -> half the PE cycles of the [feat, query]
    form.  All four (qs, h) groups of a pv pack share one psum bank with a
    single start=True zero-marking write,
  - normalizes via per-partition reciprocal scaling (free-dim broadcast),
  - transposes att [q, f] -> [f, q] via the DMA XBAR (PE + identity on the
    final block, where DMA latency would hit the critical path),
  - projects through the core's 128 Wo rows in ONE matmul (both heads
    contract together) -> a [2048, 512] bf16 partial.
Host sums the 4 partials per batch (the "all-reduce") and adds bo.

PSUM: 3 double-bank score slots (the scores->exp carousel; proj/Wo psum
tiles time-share these slots) + 2 single-bank pv packs = 8 banks.
"""

import os
import sys

sys.path.insert(0, "/opt/trn_rl_repo")

from contextlib import ExitStack

import numpy as np
import ml_dtypes

import concourse.bass as bass
import concourse.tile as tile
from concourse import bacc, mybir
from concourse.bass_utils import run_bass_kernel_spmd

BF16 = mybir.dt.bfloat16
F32 = mybir.dt.float32

B, T, D = 2, 2048, 512
H, DK = 8, 64
N_CORES = 8
P = 128  # partitions / head-pair feature count
KC = D // P  # 4 contraction chunks of 128 over d_model
NKT = T // P  # 16 key tiles of 128
NQB = 4  # query blocks
QB = T // NQB  # 512 queries per block
QSUB = QB // P  # 4 q-tiles of 128 per query block
NTC = 4  # token chunks for pipelined loads/projections

CFG = {
    "exp_bufs": 16,
    "exp2_bufs": 4,
    "att_bufs": 4,
    "out_bufs": 2,
    "ps_s_bufs": 3,
    "ps_pv_bufs": 2,
    "pv_lag": 5,
    "norm_warmup": 1,
    "q0_act_dge": 0,
    "dve_qb0_kts": (),
    "dve_qb3_extra": (),
    "split_qb0_kts": (0, 1),
    "split_last": 1,
    "pair_pso": 1,
    "v_before_q": 0,
    "q1_early_dma": 0,
    "wqkv_split": 1,
    "pv_lag0": 5,
    "last_drain_from": 11,
    "qb1_early": 3,
    "xsplit_last": 0,
    "pack_recip": 1,
    "pool_memset": 0,  # kt-lag of PV matmuls behind exp
    # exp engine split: (qb, kt) tiles with qb >= dve_min_qb and
    # kt % dve_kt_mod in dve_kt_res run on DVE as an averaged
    # double-Schraudolph (bit-trick exp2); rest on ACT (true exp)
    "dve_min_qb": 1,
    "dve_kt_mod": 16,
    "dve_kt_res": (2, 6, 9, 13),
    # pair-sum on DVE (tensor_add) vs implicit PV psum accumulation (PE)
    "pair_sum_dve": 0,
    "last_qs_dma": 0,
    "last_dma_211": 0,
    "last_dma_31": 0,
    "last_attT_alt": 0,
    "q_dma_eng": 0,
    "v_dma_eng": 0,  # final qb: one DMA per q-tile
    # out-copy engine pattern: qs indices with (qs % 2 == outcp_act_parity)
    # go to ACT; others DVE.  -1 = all DVE
    "outcp_act_parity": 1,
}
if os.environ.get("KERNEL_CFG"):
    import json

    _over = json.loads(os.environ["KERNEL_CFG"])
    for _k, _v in _over.items():
        CFG[_k] = tuple(_v) if isinstance(_v, list) else _v


def _dve_exp(qb, kt):
    if qb == 0 and kt in CFG["dve_qb0_kts"]:
        return True
    if qb == NQB - 1 and kt in CFG["dve_qb3_extra"]:
        # extra DVE tiles in the final block compress its ACT stream so the
        # last exp (which gates the whole drain chain) fires earlier
        return True
    return qb >= CFG["dve_min_qb"] and (kt % CFG["dve_kt_mod"]) in CFG["dve_kt_res"]

LOG2E = float(np.log2(np.e))
# -log2(mean of S(y)+S(y+0.5) over the sawtooth period): calibrates the
# averaged-Schraudolph pair to the same scale as ACT's true exp, so mixed
# engine tiles are consistent inside one softmax.
SCHRAUD_CAL = -1.3290842829047216


def _strided_cols(t, start, stride, count, width=1):
    """AP over columns {start + i*stride : +width} of a 2-D [P, N] tile."""
    base = t[:, start : start + width]
    # build [[part],[stride, count],[1, width]] manually
    part_dim = base.ap[0]
    esz = 1
    return bass.AP(
        tensor=base.tensor,
        offset=base.offset,
        ap=[part_dim, [stride * esz, count], [esz, width]],
    )


def _build_bass(with_bias):
    nc = bacc.Bacc(trn_type="TRN2", num_devices=N_CORES, debug=False)

    qt_d = nc.dram_tensor("qt", [D, T], BF16, kind="ExternalInput").ap()
    kt_d = nc.dram_tensor("ktin", [D, T], BF16, kind="ExternalInput").ap()
    vt_d = nc.dram_tensor("vt", [D, T], BF16, kind="ExternalInput").ap()
    # q/k/v weights arrive host-pre-swizzled as one [p, 3, c, f]
    # (partition-major) tensor so a single contiguous DMA loads all three
    wqkv_d = nc.dram_tensor("wqkv", [P, 3, KC, P], BF16, kind="ExternalInput").ap()
    wo_d = nc.dram_tensor("wo", [P, D], BF16, kind="ExternalInput").ap()
    ident_d = nc.dram_tensor("ident", [P, P], BF16, kind="ExternalInput").ap()
    if with_bias:
        bq_d = nc.dram_tensor("bq", [P, 1], F32, kind="ExternalInput").ap()
        bk_d = nc.dram_tensor("bk", [P, 1], F32, kind="ExternalInput").ap()
        bv_d = nc.dram_tensor("bv", [1, P], F32, kind="ExternalInput").ap()
    out_d = nc.dram_tensor("outp", [T, D], BF16, kind="ExternalOutput").ap()

    with tile.TileContext(nc) as tc, ExitStack() as ctx:
        singles = ctx.enter_context(tc.tile_pool(name="singles", bufs=1))
        qk_pool = ctx.enter_context(tc.tile_pool(name="qk", bufs=1))
        v_pool = ctx.enter_context(tc.tile_pool(name="vaug", bufs=NKT))
        exp_pool = ctx.enter_context(tc.tile_pool(name="exps", bufs=CFG["exp_bufs"]))
        att_pool = ctx.enter_context(tc.tile_pool(name="att", bufs=CFG["att_bufs"]))
        rden_pool = ctx.enter_context(tc.tile_pool(name="rden", bufs=4))
        out_pool = ctx.enter_context(tc.tile_pool(name="outs", bufs=CFG["out_bufs"]))
        # PSUM: scores 3*2 + pv 2*1 = 8 banks.  Proj/Wo/transpose psum
        # tiles time-share the scores slots (same tag) — they are off the
        # scores carousel's critical windows.
        ps_s = ctx.enter_context(
            tc.tile_pool(name="ps_s", bufs=CFG["ps_s_bufs"], space="PSUM")
        )
        ps_pv = ctx.enter_context(
            tc.tile_pool(name="ps_pv", bufs=CFG["ps_pv_bufs"], space="PSUM")
        )

        # ---- weight/bias loads ----
        # wqkv in thirds, interleaved with the first input slices: Wk gates
        # the k projection (the critical first-exp chain), Wq the q
        # projection, Wv only the (lagged) v projections.
        wqkv_sb = singles.tile([P, 3, KC, P], BF16)
        if CFG["wqkv_split"]:
            nc.sync.dma_start(out=wqkv_sb[:, 1], in_=wqkv_d[:, 1])
        else:
            nc.sync.dma_start(out=wqkv_sb, in_=wqkv_d)
        ident_sb = singles.tile([P, P], BF16)
        wqt_sb = wqkv_sb[:, 0]
        wkt_sb = wqkv_sb[:, 1]
        wvt_sb = wqkv_sb[:, 2]
        if with_bias:
            bq_sb = singles.tile([P, 1], F32)
            nc.sync.dma_start(out=bq_sb, in_=bq_d)
            bk_sb = singles.tile([P, 1], F32)
            nc.sync.dma_start(out=bk_sb, in_=bk_d)
            bv_sb = singles.tile([P, P], F32)
            nc.gpsimd.dma_start(
                out=bv_sb,
                in_=bass.AP(tensor=bv_d.tensor, offset=0, ap=[[0, P], [1, P]]),
            )

        # ---- chunked input loads (512-token slices) ----
        qt_sb = singles.tile([P, KC, T], BF16)
        kt_sb = singles.tile([P, KC, T], BF16)
        vt_sb = singles.tile([P, KC, T], BF16)
        ktr = kt_d.rearrange("(c p) t -> p c t", p=P)
        qtr = qt_d.rearrange("(c p) t -> p c t", p=P)
        vtr = vt_d.rearrange("(c p) t -> p c t", p=P)
        # 256-token k/q prologue slices: short enough to land early, long
        # enough (512B descriptor runs) for full DMA rate.
        nc.sync.dma_start(out=kt_sb[:, :, 0 : 2 * P], in_=ktr[:, :, 0 : 2 * P])
        if CFG["wqkv_split"]:
            nc.sync.dma_start(out=wqkv_sb[:, 0], in_=wqkv_d[:, 0])
        _q0_eng = nc.scalar if CFG["q0_act_dge"] else nc.sync
        _q0_eng.dma_start(out=qt_sb[:, :, 0 : T // NTC // 2], in_=qtr[:, :, 0 : T // NTC // 2])
        _q0_eng.dma_start(
            out=qt_sb[:, :, T // NTC // 2 : T // NTC],
            in_=qtr[:, :, T // NTC // 2 : T // NTC],
        )
        nc.sync.dma_start(
            out=kt_sb[:, :, 2 * P : T // NTC], in_=ktr[:, :, 2 * P : T // NTC]
        )
        if CFG["wqkv_split"]:
            nc.sync.dma_start(out=wqkv_sb[:, 2], in_=wqkv_d[:, 2])
        for c in range(1, NTC):
            sl = bass.ts(c, T // NTC)
            nc.sync.dma_start(out=kt_sb[:, :, sl], in_=ktr[:, :, sl])
            if c == 1 and CFG["q1_early_dma"]:
                # q chunk 1 right after k chunk 1: the qb1 tiles interleaved
                # into qb0 need it mid-way through the load stream
                nc.sync.dma_start(out=qt_sb[:, :, sl], in_=qtr[:, :, sl])
            slp = bass.ts(c - 1, T // NTC)
            if CFG["v_before_q"]:
                nc.sync.dma_start(out=vt_sb[:, :, slp], in_=vtr[:, :, slp])
                if c >= 2 and not (c == 2 and CFG["q1_early_dma"]):
                    nc.sync.dma_start(out=qt_sb[:, :, slp], in_=qtr[:, :, slp])
            else:
                if c >= 2 and not (c == 2 and CFG["q1_early_dma"]):
                    nc.sync.dma_start(out=qt_sb[:, :, slp], in_=qtr[:, :, slp])
                nc.sync.dma_start(out=vt_sb[:, :, slp], in_=vtr[:, :, slp])
        slz = bass.ts(NTC - 1, T // NTC)
        if CFG["v_before_q"]:
            nc.sync.dma_start(out=vt_sb[:, :, slz], in_=vtr[:, :, slz])
            nc.sync.dma_start(out=qt_sb[:, :, slz], in_=qtr[:, :, slz])
        else:
            nc.sync.dma_start(out=qt_sb[:, :, slz], in_=qtr[:, :, slz])
            nc.sync.dma_start(out=vt_sb[:, :, slz], in_=vtr[:, :, slz])
        # ident/Wo are not needed until the first qb tail — load last.
        nc.sync.dma_start(out=ident_sb, in_=ident_d)
        wo_sb = singles.tile([P, D], BF16)
        nc.sync.dma_start(out=wo_sb, in_=wo_d)

        # ---- projections + attention ----
        qT = qk_pool.tile([P, T], BF16)
        kT = qk_pool.tile([P, T], BF16)
        v_aug = [None] * NKT
        inv_sqrt_dk = float(1.0 / np.sqrt(DK))

        def emit_qk_proj(dst, src_sb, w_sb, b_sb, c, lo=None, cs=None, psq=None):
            if cs is None:
                cs = T // NTC
            sl = bass.ds(c * (T // NTC) if lo is None else lo, cs)
            if psq is None:
                psq = ps_s.tile([P, 2 * QB], F32, tag="scores", name="psq")
            for kc in range(KC):
                nc.tensor.matmul(
                    psq[:, 0:cs],
                    w_sb[:, kc, :],
                    src_sb[:, kc, sl],
                    start=(kc == 0),
                    stop=(kc == KC - 1),
                )
            nc.vector.tensor_copy(dst[:, sl], psq[:, 0:cs])
            if b_sb is not None:
                nc.vector.tensor_add(
                    dst[:, sl], dst[:, sl], b_sb[:, :].broadcast_to([P, cs])
                )

        def emit_v_proj(kt0):
            # project TWO k-tiles into one psum tile (fewer carousel slots)
            psv = ps_s.tile([P, 2, P], F32, tag="scores", name="psv")
            for j in range(2):
                for kc in range(KC):
                    nc.tensor.matmul(
                        psv[:, j, :],
                        vt_sb[:, kc, bass.ts(kt0 + j, P)],
                        wvt_sb[:, kc, :],
                        start=(kc == 0 and j == 0),
                        stop=(kc == KC - 1 and j == 1),
                        skip_group_check=True,
                    )
            for j in range(2):
                # layout [vA(0:64) | 1 | vB(65:129) | 1]: per-head moving
                # slices of 65 columns are contiguous
                va = v_pool.tile([P, 2 * DK + 2], BF16, tag="vaug")
                dst = _strided_cols(va, 0, DK + 1, 2, width=DK)
                nc.vector.tensor_copy(
                    dst, psv[:, j, :].rearrange("p (h f) -> p h f", h=2)
                )
                if with_bias:
                    nc.vector.tensor_add(
                        dst, dst, bv_sb[:, :].rearrange("p (h f) -> p h f", h=2)
                    )
                if CFG["pool_memset"]:
                    nc.gpsimd.memset(_strided_cols(va, DK, DK + 1, 2), 1.0)
                else:
                    nc.vector.memset(_strided_cols(va, DK, DK + 1, 2), 1.0)
                v_aug[kt0 + j] = va

        def emit_pv(pvs, kt, exs):
            # [q, f] orientation: stationary = ex q-tile, moving = v_aug head
            # slice.  All four (qs, h) groups of a pv pack share one psum
            # bank: only the very first write uses start=True (zero-marks
            # the whole 2KB region); everything else accumulates.  DVE-exp
            # tiles pass TWO ex halves (Schraudolph pair) — psum accumulation
            # performs the pair-sum, keeping the add off the critical path.
            for qs in range(QSUB):
                pv = pvs[qs // 2][:, qs % 2]
                for h in range(2):
                    for xi, ex in enumerate(exs):
                        nc.tensor.matmul(
                            pv[:, h * (DK + 1) : (h + 1) * (DK + 1)],
                            ex[:, h, bass.ts(qs, P)],
                            v_aug[kt][:, h * (DK + 1) : (h + 1) * (DK + 1)],
                            start=(kt == 0 and qs % 2 == 0 and h == 0 and xi == 0),
                            stop=(kt == NKT - 1 and xi == len(exs) - 1),
                            skip_group_check=True,
                        )

        I16 = mybir.dt.int16
        sch_a = float(inv_sqrt_dk * LOG2E * 128.0)
        sch_b = float(128.0 * (127.0 + SCHRAUD_CAL))

        def emit_attn_group(qb, pvs, kts, pending, lag=True, split=False):
            # software-pipelined: PV(kt) is emitted AFTER QK/exp(kt+lag) so
            # the scalar engine (the bottleneck) is never starved.  split=True
            # emits scores/exp per query-half so the first exp can fire as
            # soon as the first half of qT chunk 0 is projected.
            for kt in kts:
                pss = ps_s.tile([P, 2, QB], F32, tag="scores")
                tile_split = bool(
                    split
                    or (CFG["split_last"] and qb == NQB - 1 and kt == NKT - 1)
                )
                nhv = 2 if tile_split else 1
                for hv in range(nhv):
                    qsl = bass.ds(qb * QB + hv * (QB // nhv), QB // nhv)
                    csl = bass.ds(hv * (QB // nhv), QB // nhv)
                    nc.tensor.matmul(
                        pss[:, 0, csl],
                        kT[0:DK, bass.ts(kt, P)],
                        qT[0:DK, qsl],
                        start=(hv == 0),
                        stop=(hv == nhv - 1),
                        skip_group_check=tile_split,
                    )
                    nc.tensor.matmul(
                        pss[:, 1, csl],
                        kT[DK:P, bass.ts(kt, P)],
                        qT[DK:P, qsl],
                        start=(hv == 0),
                        stop=(hv == nhv - 1),
                        skip_group_check=tile_split,
                    )
                ex = exp_pool.tile([P, 2, QB], BF16, tag="exps")
                if _dve_exp(qb, kt):
                    # DVE averaged double-Schraudolph: ex-pair = S(y), S(y+0.5)
                    # where S maps the log2-domain score to bf16 bits via a
                    # saturating f32->int16 round.  S(y+0.5) is exactly
                    # bits+64, and the pair's constant mean is folded into
                    # sch_b so these tiles match ACT's true-exp scale.  The
                    # pair is summed implicitly by PV psum accumulation.
                    exb = exp_pool.tile([P, 2, QB], BF16, tag="exps2", bufs=CFG["exp2_bufs"])
                    nc.vector.tensor_scalar(
                        out=ex.bitcast(I16),
                        in0=pss,
                        scalar1=sch_a,
                        scalar2=sch_b,
                        op0=mybir.AluOpType.mult,
                        op1=mybir.AluOpType.add,
                    )
                    nc.vector.tensor_scalar(
                        out=exb.bitcast(I16),
                        in0=ex.bitcast(I16),
                        scalar1=64,
                        scalar2=None,
                        op0=mybir.AluOpType.add,
                    )
                    if CFG["pair_sum_dve"]:
                        exs_t = exp_pool.tile([P, 2, QB], BF16, tag="exps3", bufs=CFG["exp2_bufs"])
                        nc.vector.tensor_add(exs_t, ex, exb)
                        exs = (exs_t,)
                    else:
                        exs = (ex, exb)
                else:
                    if tile_split:
                        xs = (
                            CFG["xsplit_last"]
                            and qb == NQB - 1
                            and kt == NKT - 1
                        )
                        for hv in range(2):
                            csl = bass.ds(hv * (QB // 2), QB // 2)
                            if xs and hv == 1:
                                # final tile: second query-half via the DVE
                                # Schraudolph pair, concurrent with ACT's
                                # first half - the drain chain starts sooner
                                exb = exp_pool.tile(
                                    [P, 2, QB], BF16, tag="exps2",
                                    bufs=CFG["exp2_bufs"], name="exb_x",
                                )
                                nc.vector.tensor_scalar(
                                    out=ex.bitcast(I16)[:, :, csl],
                                    in0=pss[:, :, csl],
                                    scalar1=sch_a,
                                    scalar2=sch_b,
                                    op0=mybir.AluOpType.mult,
                                    op1=mybir.AluOpType.add,
                                )
                                nc.vector.tensor_scalar(
                                    out=exb.bitcast(I16)[:, :, csl],
                                    in0=ex.bitcast(I16)[:, :, csl],
                                    scalar1=64,
                                    scalar2=None,
                                    op0=mybir.AluOpType.add,
                                )
                                nc.vector.tensor_add(
                                    ex[:, :, csl], ex[:, :, csl], exb[:, :, csl]
                                )
                            else:
                                nc.scalar.activation(
                                    out=ex[:, :, csl],
                                    in_=pss[:, :, csl],
                                    func=mybir.ActivationFunctionType.Exp,
                                    scale=inv_sqrt_dk,
                                )
                    else:
                        nc.scalar.activation(
                            out=ex,
                            in_=pss,
                            func=mybir.ActivationFunctionType.Exp,
                            scale=inv_sqrt_dk,
                        )
                    exs = (ex,)
                lag_now = CFG["pv_lag0"] if qb == 0 else CFG["pv_lag"]
                if qb == NQB - 1 and kt >= CFG["last_drain_from"]:
                    lag_now = 1
                while len(pending) >= lag_now:
                    emit_pv(*pending.pop(0))
                if lag:
                    pending.append((pvs, kt, exs))
                else:
                    emit_pv(pvs, kt, exs)

        def emit_qb_norms(qb, pvs):
            """Urgent tail half: normalize att and launch the DMA transposes.
            Frees the pv psum packs for the next qb; must be emitted BEFORE
            the next qb's attn group so it precedes the next qb's DVE work
            in queue order."""
            last = qb == NQB - 1
            atts = []
            r4s = []
            if CFG["pack_recip"]:
                for pk in range(2):
                    r4 = rden_pool.tile([P, 4], F32, tag="rden")
                    nc.vector.reciprocal(
                        r4, _strided_cols(pvs[pk].rearrange("p a b -> p (a b)"), DK, DK + 1, 4)
                    )
                    r4s.append(r4)
            for qs in range(QSUB):
                pv = pvs[qs // 2][:, qs % 2]  # [128, 130]
                if CFG["pack_recip"]:
                    r2 = r4s[qs // 2][:, 2 * (qs % 2) : 2 * (qs % 2) + 2]
                else:
                    r2 = rden_pool.tile([P, 2], F32, tag="rden")
                    nc.vector.reciprocal(r2, _strided_cols(pv, DK, DK + 1, 2))
                att = att_pool.tile([P, P], BF16, tag="att")
                for h in range(2):
                    nc.vector.tensor_scalar(
                        out=att[:, h * DK : (h + 1) * DK],
                        in0=pv[:, h * (DK + 1) : h * (DK + 1) + DK],
                        scalar1=r2[:, h : h + 1],
                        scalar2=None,
                        op0=mybir.AluOpType.mult,
                    )
                attT = att_pool.tile([P, P], BF16, tag="attT")
                if last or CFG.get("pe_transpose_all", 0):
                    attT_ps = ps_s.tile([P, P], BF16, tag="scores", name="attT_ps")
                    nc.tensor.transpose(attT_ps, att, ident_sb)
                    nc.scalar.copy(attT, attT_ps)
                else:
                    nc.sync.dma_start_transpose(out=attT, in_=att)
                atts.append(attT)
            return atts

        def emit_qb_out(qb, atts):
            """Deferred tail half: Wo projection + psum->sbuf copies + DMA.
            Non-final qbs pair two q-tiles per Wo psum tile (one carousel
            slot, one copy) to cut slot churn; the final qb keeps per-qs
            granularity for the shortest path to the last byte."""
            last = qb == NQB - 1
            if CFG["pair_pso"] and not last:
                for half in range(2):
                    out_sb = out_pool.tile([P, 2, D], BF16, tag="outs")
                    pso = ps_s.tile([P, 2, D], F32, tag="scores", name="pso")
                    for i in range(2):
                        qs = 2 * half + i
                        # each [:, i, :] half is its own full psum bank and
                        # accumulation group: both need start=True
                        nc.tensor.matmul(
                            pso[:, i, :], atts[qs], wo_sb,
                            start=True, stop=True,
                        )
                    if half % 2 == CFG["outcp_act_parity"]:
                        nc.scalar.copy(out_sb, pso)
                    else:
                        nc.vector.tensor_copy(out_sb, pso)
                    nc.sync.dma_start(
                        out=out_d[bass.ds(qb * QB + half * 2 * P, 2 * P), :].rearrange(
                            "(c p) o -> p c o", p=P
                        ),
                        in_=out_sb,
                    )
                return
            if last and CFG["last_dma_31"]:
                groups = [(0, 1, 2), (3,)]
            elif last and CFG["last_dma_211"]:
                groups = [(0, 1), (2,), (3,)]
            elif last and CFG["last_qs_dma"]:
                groups = [(0,), (1,), (2,), (3,)]
            else:
                groups = [(0, 1), (2, 3)]
            for grp in groups:
                out_sb = out_pool.tile([P, len(grp), D], BF16, tag="outs")
                for i, qs in enumerate(grp):
                    pso = ps_s.tile([P, D], F32, tag="scores", name="pso")
                    nc.tensor.matmul(pso, atts[qs], wo_sb, start=True, stop=True)
                    if CFG["outcp_act_parity"] >= 0 and qs % 2 == CFG["outcp_act_parity"]:
                        nc.scalar.copy(out_sb[:, i, :], pso)
                    else:
                        nc.vector.tensor_copy(out_sb[:, i, :], pso)
                nc.sync.dma_start(
                    out=out_d[bass.ds(qb * QB + grp[0] * P, len(grp) * P), :].rearrange(
                        "(c p) o -> p c o", p=P
                    ),
                    in_=out_sb,
                )

        # qb0 is interleaved with the per-chunk projections so the scalar
        # engine (softmax exp — the bottleneck) starts as early as possible.
        bqs = bq_sb if with_bias else None
        bks = bk_sb if with_bias else None

        def alloc_pvs():
            # two packs of [128, 2 qs-slots, 130] f32; each pack = 1 psum bank
            pvpack_a = ps_pv.tile([P, 2, 2 * DK + 2], F32, tag="pv", name="pvpack_a")
            pvpack_b = ps_pv.tile([P, 2, 2 * DK + 2], F32, tag="pv", name="pvpack_b")
            return [pvpack_a, pvpack_b]

        pvs0 = alloc_pvs()
        kpc = NKT // NTC  # k-tiles per chunk
        pending = []
        emit_qk_proj(kT, kt_sb, wkt_sb, bks, 0, lo=0, cs=2 * P)
        emit_qk_proj(qT, qt_sb, wqt_sb, bqs, 0, lo=0, cs=T // NTC // 2)
        emit_qk_proj(qT, qt_sb, wqt_sb, bqs, 0, lo=T // NTC // 2, cs=T // NTC // 2)
        emit_attn_group(0, pvs0, [0], pending, split=0 in CFG["split_qb0_kts"])
        emit_qk_proj(kT, kt_sb, wkt_sb, bks, 0, lo=2 * P, cs=T // NTC - 2 * P)
        pvs1 = None
        n1e = CFG["qb1_early"]
        for c in range(NTC):
            # attn kts of chunk c first (alloc priority for the exp
            # carousel); this chunk's v-projections interleave mid-chunk so
            # their psum slots only enter the carousel around the time the
            # v chunk's DMA lands.  qb1's first tiles interleave into qb0's
            # later (DMA-gated) chunks to fill ACT's chunk-arrival gaps.
            kl = list(range(1, kpc) if c == 0 else range(c * kpc, (c + 1) * kpc))
            emit_attn_group(0, pvs0, kl[:1], pending, split=kl[0] in CFG["split_qb0_kts"])
            emit_v_proj(c * kpc)
            emit_attn_group(0, pvs0, kl[1:2], pending)
            emit_v_proj(c * kpc + 2)
            emit_attn_group(0, pvs0, kl[2:], pending)
            if c + 1 < NTC:
                emit_qk_proj(kT, kt_sb, wkt_sb, bks, c + 1)
            if c == 1 and n1e:
                emit_qk_proj(qT, qt_sb, wqt_sb, bqs, 1)
            if c == 2 and n1e:
                pvs1 = alloc_pvs()
                emit_attn_group(1, pvs1, range(n1e), pending)
        for c in range(2 if n1e else 1, NTC):
            emit_qk_proj(qT, qt_sb, wqt_sb, bqs, c)
        prev_qb, prev_atts = 0, None
        warm = CFG["norm_warmup"]  # attn tiles of qb n+1 before qb n's norms

        def drain_for(pvs_of):
            # emit every lagged PV group belonging to pvs_of (keep FIFO
            # order for the rest; pending can hold interleaved qbs)
            rest = []
            while pending:
                g = pending.pop(0)
                if g[0] is pvs_of:
                    emit_pv(*g)
                else:
                    rest.append(g)
            pending.extend(rest)

        for qb in range(1, NQB):
            prev_pvs = pvs0 if prev_qb == 0 else pvs
            start_kt = n1e if (qb == 1 and n1e) else 0
            pvs = pvs1 if (qb == 1 and n1e) else alloc_pvs()
            last = qb == NQB - 1
            emit_attn_group(qb, pvs, range(start_kt, start_kt + warm), pending, lag=True)
            # the previous qb's lagged PV groups must land before its norms
            drain_for(prev_pvs)
            prev_atts = emit_qb_norms(prev_qb, prev_pvs)
            emit_attn_group(qb, pvs, range(start_kt + warm, NKT), pending, lag=True)
            if last:
                while pending:
                    emit_pv(*pending.pop(0))
            emit_qb_out(prev_qb, prev_atts)
            prev_qb = qb
        atts = emit_qb_norms(prev_qb, pvs)
        emit_qb_out(prev_qb, atts)

    nc.compile()
    return nc


_NC_CACHE = {}


def _get_nc(with_bias):
    if with_bias not in _NC_CACHE:
        _NC_CACHE[with_bias] = _build_bass(with_bias)
    return _NC_CACHE[with_bias]


def _prep_in_maps(Q, K, V, Wq, bq, Wk, bk, Wv, bv, Wo, bo, with_bias):
    bf = ml_dtypes.bfloat16
    f32 = np.float32
    qkvT = []  # per batch: transposed bf16 [D, T]
    for X in (Q, K, V):
        qkvT.append([np.ascontiguousarray(X[b].T.astype(bf)) for b in range(B)])
    woT = np.ascontiguousarray(Wo.T.astype(bf))  # [D feat, D out]
    ident = np.eye(P, dtype=bf)

    def swz(w_rows):  # [P, D] slice of W -> transposed+partition-major [P, KC, P]
        return np.ascontiguousarray(
            w_rows.T.astype(bf).reshape(KC, P, P).transpose(1, 0, 2)
        )
    in_maps = []
    for c in range(N_CORES):
        b, p = divmod(c, 4)
        rows = slice(P * p, P * (p + 1))
        m = {
            "qt": qkvT[0][b],
            "ktin": qkvT[1][b],
            "vt": qkvT[2][b],
            "wqkv": np.ascontiguousarray(
                np.stack([swz(Wq[rows]), swz(Wk[rows]), swz(Wv[rows])], axis=1)
            ),
            "wo": np.ascontiguousarray(woT[rows]),
            "ident": ident,
        }
        if with_bias:
            m["bq"] = np.ascontiguousarray(bq[rows].astype(f32).reshape(P, 1))
            m["bk"] = np.ascontiguousarray(bk[rows].astype(f32).reshape(P, 1))
            m["bv"] = np.ascontiguousarray(bv[rows].astype(f32).reshape(1, P))
        in_maps.append(m)
    return in_maps


def kernel(Q, K, V, Wq, bq, Wk, bk, Wv, bv, Wo, bo, _return_raw=False):
    # accept jax arrays / lists transparently
    Q, K, V = np.asarray(Q), np.asarray(K), np.asarray(V)
    Wq, Wk, Wv, Wo = (np.asarray(x) for x in (Wq, Wk, Wv, Wo))
    bq, bk, bv, bo = (np.asarray(x) for x in (bq, bk, bv, bo))
    with_bias = bool(np.any(bq) or np.any(bk) or np.any(bv))
    nc = _get_nc(with_bias)
    in_maps = _prep_in_maps(Q, K, V, Wq, bq, Wk, bk, Wv, bv, Wo, bo, with_bias)
    try:
        res = run_bass_kernel_spmd(
            nc,
            in_maps,
            core_ids=list(range(N_CORES)),
            trace=os.environ.get("KERNEL_TRACE", "0") == "1",
        )
    except ModuleNotFoundError:
        # BASS_TRACE was requested but this axon build lacks the NTFF
        # profile hook (antenv.axon_hooks) — rerun with tracing disabled.
        os.environ["BASS_NEVER_TRACE"] = "1"
        res = run_bass_kernel_spmd(
            nc, in_maps, core_ids=list(range(N_CORES)), trace=False
        )
    parts = [
        np.asarray(r["outp"]).view(ml_dtypes.bfloat16).astype(np.float32)
        if np.asarray(r["outp"]).dtype != np.float32
        and np.asarray(r["outp"]).dtype != ml_dtypes.bfloat16
        else np.asarray(r["outp"], dtype=np.float32)
        for r in res.results
    ]
    out = np.empty((B, T, D), np.float32)
    for b in range(B):
        out[b] = parts[4 * b]
        for p in range(1, 4):
            out[b] += parts[4 * b + p]
        out[b] += bo.astype(np.float32)
    if _return_raw:
        return out, res
    return out

